# revision 31
# baseline (speedup 1.0000x reference)
"""MultiOutSizeLinear (MoE-style routed linear) for Trainium2, 8 NeuronCores.

Each token selects one of 4 experts by its ``out_feat_size`` value
(128/256/512/1024). Expert k is a dense [out_k, 1024] linear + bias whose
output lands in the first out_k columns of the 1024-wide output row; the
reference leaves bias[k, out_k:] in the remaining columns (zero for the
shipped setup_inputs, which pre-zeroes the bias tail).

Strategy
  host:   route tokens to experts; balance each expert's tokens evenly
          across the 8 cores (capacities are shared so one SPMD program
          serves all cores); gather + transpose each core's tokens into
          x^T [1024, TPAD] laid out as expert segments [e3 | e2 | e1 | e0],
          cast to bfloat16 (fp32 PSUM accumulation keeps rel err ~1e-3).
  device: keep W^T [1024, 1920] (all experts, concatenated out-columns),
          bias tiles, small per-expert output staging buffers, and -- SBUF
          permitting, which it is for the shipped shapes -- the ENTIRE
          per-core x^T resident in SBUF, so the steady-state body issues
          ZERO read DMAs.

          Single-shot schedule (what a one-shot profile sees):
          - Prologue DMAs are issued just-in-time on the SP HWDGE ring:
            [wt(first expert) ⊕ x group 0, per ktile] bb bbr wt(2nd) x1 x2
            x3 wt(3rd) ... so the first matmul waits on ~0.4 MB, not 5 MB
            (head stall 19 us -> 3 us modeled).
          - Output flushes ride the Activation-engine HWDGE ring in
            0.13-0.5 MB chunks, so they never sit FIFO-behind prologue x
            loads (that stalled the PSUM recycle chain ~11 us) and the
            final drain tail is ~1 us.
          - Segments run big-expert-first [e3|e2|e1|e0]: e3's per-group PE
            time is the largest, so the x stream stays ahead of compute.

          PE schedule: HW measures ~0.63-0.74 ns/col sustained (power
          throttle off the 2.4 GHz nominal; drifts with load history) plus
          ~26 ns fixed cost per matmul instruction, so the instruction
          count matters as much as streamed columns:
          - e2/e3 token-stationary: psum[128 tok, out_k] += xT.T @ wT, kk
            OUTER / 512-col chunk inner, so e3's consecutive matmuls reuse
            the stationary x tile (~17 ns cheaper than swapping).
          - e0/e1 weight-stationary (WS_SMALL): stationary = w tile
            [128 k, 128 out-cols], moving = x [128 k, <=512 tokens] ->
            psum[128 out-col, tok]; 96 matmuls of 512-wide streams replace
            256 matmuls of 128/256-wide ones. Output is transposed
            ([out_k, cap]); the host untransposes for free. Bias comes
            from per-partition f32 columns (``bbt``) via tensor_scalar_add.
          Bias for e2/e3 is added on VectorE during PSUM eviction.
  host:   scatter rows back through the routing permutation (upcast to f32).

Measured (8-core SPMD, loop-delta steady state): ~177-210 us depending on
the chip's sustained-throttle state; CoreSim single-shot model 121 us at
2.4 GHz / 173 us at the observed 1.55 GHz sustained rate (baseline kernel:
147.5 / 191.3). fp8 DoubleRow was measured 2x bf16 on HW but plain-fp8
accuracy (rel ~3.8e-2) fails the 2e-2 gate and any error-compensation term
costs full moving-port bandwidth, erasing the speedup -- so bf16 it is.
"""

import sys
import numpy as np

sys.path.insert(0, "/opt/trn_rl_repo")

OUT_SIZES = (128, 256, 512, 1024)
N_EXPERTS = len(OUT_SIZES)
IN_FEAT = 1024
N_CORES = 8
K_TILES = IN_FEAT // 128
BLK = 512       # tokens per x^T DRAM block
XGRP = 1        # 512-token blocks per x dma_start
CHUNK = BLK     # kept name: host DRAM layout is BLK-blocked
WOFF = tuple(int(np.cumsum((0,) + OUT_SIZES)[k]) for k in range(N_EXPERTS))
W_COLS = sum(OUT_SIZES)
# Big expert first: its per-group PE time is the largest, so the x-group
# stream on the SP ring never starves the compute cursor; the smaller
# experts run at the end when all of x is long resident. (Small-first
# reads ~200 KB before the first matmul but starves mid-kernel: modeled
# 198 us vs 174 us at the sustained HW clock.)
SEG_ORDER = (3, 2, 1, 0)   # segment order along the token axis
KK_OUTER = True            # kk outer / col-chunk inner for >512-wide experts
FLUSH_FINE = True          # fine-grained output flushes vs half-segment
FLUSH_ACT = True           # output flushes on the Activation HWDGE ring
WS_SMALL = True            # weight-stationary path for experts <=256 wide:
                           # 512-token moving streams instead of 128/256-col
                           # ones cut the matmul count 256 -> 96 (~26 ns of
                           # fixed issue cost each); output comes out
                           # transposed [ok, cap] and is fixed up on host

_nc_cache: dict = {}


def _build(caps, repeat=1, loop=None, xbufs=3, obufs=None,
           drop_out=False, fake_x=False, resident_groups="auto",
           read_lead=None, evict="add"):
    """Compile the SPMD program for shared per-expert capacities ``caps``.

    caps[k] % 128 == 0; computed tokens sum(caps) need not be 512-aligned
    (the final DRAM block is padded). ``repeat``/``loop`` re-run the
    compute body (same I/O) for timing.
    """
    import concourse.bacc as bacc
    import concourse.mybir as mybir
    import concourse.tile as tile

    f32 = mybir.dt.float32
    bf16 = mybir.dt.bfloat16
    ws_set = {k for k in range(N_EXPERTS)
              if WS_SMALL and OUT_SIZES[k] <= 256 and caps[k]}
    tpad = sum(caps)
    assert all(c % 128 == 0 for c in caps)
    nblocks = -(-tpad // BLK)          # DRAM blocks (last may be partial)
    ngroups = -(-nblocks // XGRP)

    if resident_groups == "auto":
        # resident x^T groups: whatever SBUF has left after the weights,
        # bias, and per-expert half-segment output buffers. When the whole
        # x^T fits (the common case), the body has ZERO read DMAs and the
        # streaming rotation pool is never used.
        grp_b = XGRP * K_TILES * BLK * 2
        fixed = (K_TILES * W_COLS * 2 + W_COLS * 2
                 + sum(-(-(caps[k] // 128) // 2) * OUT_SIZES[k] * 2
                       for k in range(N_EXPERTS)))
        if fixed + ngroups * grp_b <= 206 * 1024:
            resident_groups = ngroups
        else:
            resident_groups = max(0, (200 * 1024 - fixed - xbufs * grp_b)
                                  // grp_b)

    nc = bacc.Bacc(None, target_bir_lowering=False, debug=False)
    # block-layout x^T: block c holds tokens [c*BLK, (c+1)*BLK) as a
    # contiguous [IN_FEAT, BLK] slab; one dma_start covers XGRP blocks
    xt = nc.dram_tensor("xt", [nblocks, IN_FEAT, BLK], bf16,
                        kind="ExternalInput")
    wt = nc.dram_tensor("wt", [IN_FEAT, W_COLS], bf16, kind="ExternalInput")
    bb = nc.dram_tensor("bb", [128, W_COLS], bf16, kind="ExternalInput")
    # replicated bias for packed evictions: cols [0,512) = bias0 x4,
    # cols [512,1024) = bias1 x2
    bbr = nc.dram_tensor("bbr", [128, 1024], bf16, kind="ExternalInput")
    # per-partition bias columns for the weight-stationary experts:
    # bbt[p, wsj_col[(k, j)]] = bias[k, j*128 + p]
    wsj_col = {}
    for k in sorted(ws_set):
        for j in range(OUT_SIZES[k] // 128):
            wsj_col[(k, j)] = len(wsj_col)
    bbt = (nc.dram_tensor("bbt", [128, len(wsj_col)], f32,
                          kind="ExternalInput") if wsj_col else None)
    outs = {k: nc.dram_tensor(
                f"out{k}",
                ([OUT_SIZES[k], caps[k]] if k in ws_set
                 else [caps[k], OUT_SIZES[k]]), bf16,
                kind="ExternalOutput")
            for k in range(N_EXPERTS) if caps[k]}

    # smallest expert first: its weight slice + first x tiles are a few
    # hundred KB, so the PE starts ~2 us into the kernel instead of waiting
    # for the big experts' multi-MB weight slabs; each later expert's wt
    # chunk is DMA'd while earlier segments compute
    seg_order = [k for k in SEG_ORDER if caps[k] > 0]
    seg_start = {}
    t0 = 0
    for k in seg_order:
        seg_start[k] = t0
        t0 += caps[k]

    def expert_of(tok):
        for k in seg_order:
            if tok < seg_start[k] + caps[k]:
                return k
        raise AssertionError

    with tile.TileContext(nc) as tc:
        with (
            tc.tile_pool(name="const", bufs=1) as const,
            tc.tile_pool(name="xp", bufs=xbufs) as xp,
            tc.tile_pool(name="ps", bufs=4, space="PSUM") as psp,
        ):
            # Prologue DMA order is just-in-time for the compute schedule:
            #   [wt(e_first) ⊕ x group 0 per-ktile] bb bbr wt(e2nd) x1 x2 x3
            #   wt(e3rd) x4 x5 wt(e4th) x6 ... x16
            # so the first matmul waits on ~0.2 MB, and every later expert's
            # weight slice / x group lands before its segment needs it.
            wt_sb = const.tile([128, K_TILES, W_COLS], bf16)
            xr0 = None
            g0 = tk0 = 0
            if resident_groups != 0 and ngroups > 0:
                xr0 = const.tile([128, XGRP, K_TILES, BLK], bf16, name="xres0")
                g0 = min(XGRP, nblocks)
                tk0 = BLK if (nblocks > 1 or tpad % BLK == 0) else tpad % BLK
            k1 = seg_order[0]
            for kk in range(K_TILES):
                nc.sync.dma_start(
                    wt_sb[:, kk, WOFF[k1]:WOFF[k1] + OUT_SIZES[k1]],
                    wt[kk * 128:(kk + 1) * 128,
                       WOFF[k1]:WOFF[k1] + OUT_SIZES[k1]])
                if xr0 is not None:
                    for ci in range(g0):
                        tk = tk0 if ci == g0 - 1 else BLK
                        nc.sync.dma_start(
                            xr0[:, ci, kk, :tk],
                            xt[ci, kk * 128:(kk + 1) * 128, :tk])
            bb_sb = const.tile([128, W_COLS], bf16)
            nc.sync.dma_start(bb_sb[:], bb[:])
            bbr_sb = const.tile([128, 1024], bf16)
            nc.sync.dma_start(bbr_sb[:], bbr[:])
            bbr_off = {0: 0, 1: 512}

            def load_wt_expert(k):
                for kk in range(K_TILES):
                    nc.sync.dma_start(
                        wt_sb[:, kk, WOFF[k]:WOFF[k] + OUT_SIZES[k]],
                        wt[kk * 128:(kk + 1) * 128,
                           WOFF[k]:WOFF[k] + OUT_SIZES[k]])
            # remaining experts' weights interleave with early x groups:
            # issued before resident groups 1, 3, 5, ... (each arrives well
            # before its segment's first block)
            wt_before_group = {1 + 2 * i: k
                               for i, k in enumerate(seg_order[1:])}

            # half-segment output buffers: DVE evicts into slot bi % half;
            # flushed as ONE DMA per half per expert per iteration (the
            # 4-deep PSUM rotation absorbs the WAR wait when the second
            # half starts while the first half's flush drains)
            o_seg = {}
            o_half = {}
            odt = mybir.dt.float32 if evict == "addf32" else bf16
            for k in seg_order:
                if k in ws_set:
                    continue
                nblk_k = caps[k] // 128
                o_half[k] = 4 if drop_out else -(-nblk_k // 2)
                o_seg[k] = const.tile([128, o_half[k], OUT_SIZES[k]],
                                      odt, name=f"oseg{k}")
            # weight-stationary experts: transposed [out-col part, token]
            # chunk buffers, double-buffered against the flush DMA
            o_ws = {k: const.tile([128, 2, (OUT_SIZES[k] // 128) * 512],
                                  odt, name=f"ows{k}")
                    for k in sorted(ws_set)}
            bbt_sb = None
            if wsj_col:
                bbt_sb = const.tile([128, len(wsj_col)], f32)
                nc.sync.dma_start(bbt_sb[:], bbt[:])

            # blocks packed per PSUM tile (ok*pack <= 512): batches expert
            # 0/1 evictions so the PE->DVE->PE psum-recycle sem chain fires
            # 4x/2x less often in the fast small-expert tail
            packs = {}
            for k in seg_order:
                if k in ws_set:
                    packs[k] = 1
                    continue
                nblk_k = caps[k] // 128
                p = 1
                if k in bbr_off:
                    for cand in (4, 2):
                        if (cand * OUT_SIZES[k] <= 512
                                and nblk_k % cand == 0
                                and o_half[k] % cand == 0):
                            p = cand
                            break
                packs[k] = p

            def xsrc(gi):
                """(blocks, tokens-in-last-block, src AP) for x group gi.
                The final DRAM block is only read up to the last computed
                token."""
                s = gi * XGRP
                g = min(XGRP, nblocks - s)
                tk = BLK
                if s + g == nblocks and tpad % BLK and g == 1:
                    tk = tpad % BLK
                    src = xt[s:s + 1, :, :tk].rearrange(
                        "c (kk p) t -> p c kk t", p=128)
                else:
                    src = xt[s:s + g].rearrange("c (kk p) t -> p c kk t", p=128)
                return g, tk, src

            # leading groups of x^T stay resident: each loop iteration
            # starts computing on them immediately (loaded once, before the
            # loop), and they are not re-read per iteration. Group 0 was
            # already issued (per-ktile, interleaved with wt) above.
            nres = min(resident_groups, ngroups)
            xres = {}
            if xr0 is not None and nres > 0:
                xres[0] = xr0
            for gi in range(1, nres):
                if gi in wt_before_group:
                    load_wt_expert(wt_before_group[gi])
                gsz, tk, src = xsrc(gi)
                xr = const.tile([128, XGRP, K_TILES, BLK], bf16,
                                name=f"xres{gi}")
                nc.sync.dma_start(xr[:, :gsz, :, :tk], src)
                xres[gi] = xr
            # wt slices not yet issued (few/zero resident groups)
            for gi, k in wt_before_group.items():
                if gi >= max(nres, 1):
                    load_wt_expert(k)

            # flush cadence (blocks): >=2 KB per partition line per DMA,
            # always a multiple of the pack size, and <= the slot count so
            # a flush is issued before any o_seg slot is reused
            FB = {k: min(max(packs[k], 2048 // (OUT_SIZES[k] * 2) or 1),
                         o_half[k])
                  for k in seg_order if k not in ws_set}

            def body():
                xtiles = dict(xres)
                fl_next = {k: 0 for k in seg_order}
                ws_chunk = {k: 0 for k in ws_set}

                def issue_x(gi):
                    if gi < nres or gi >= ngroups or fake_x:
                        return
                    g, tk, src = xsrc(gi)
                    x_sb = xp.tile([128, XGRP, K_TILES, BLK], bf16, tag="x")
                    nc.sync.dma_start(x_sb[:, :g, :, :tk], src)
                    xtiles[gi] = x_sb

                if read_lead is None:
                    for gi in range(nres, ngroups):
                        issue_x(gi)
                else:
                    for gi in range(nres, min(nres + read_lead, ngroups)):
                        issue_x(gi)

                next_read = nres + (read_lead or 0)
                cur_ps = None
                for tok in range(0, tpad, 128):
                    k = expert_of(tok)
                    ok = OUT_SIZES[k]
                    P = packs[k]
                    blk, off = divmod(tok, BLK)
                    gi, ci = divmod(blk, XGRP)
                    if read_lead is not None and off == 0 and ci == 0:
                        # compute cursor entered group gi: top up the lead
                        while next_read <= gi + read_lead and next_read < ngroups:
                            issue_x(next_read)
                            next_read += 1
                    x_sb = xtiles[0 if fake_x else gi]
                    if k in ws_set:
                        # weight-stationary: one [128-out-col, T<=512-token]
                        # chunk per psum group, streaming tokens as the
                        # moving operand. Emitted once per chunk (chunks
                        # split at xt-block boundaries); other 128-blocks
                        # inside the chunk are skipped.
                        tloc = tok - seg_start[k]
                        if tloc != 0 and off != 0:
                            continue
                        T = min(BLK - off, caps[k] - tloc)
                        nj = ok // 128
                        psw = psp.tile([128, 1024], f32, tag="ps")
                        for kk in range(K_TILES):
                            for j in range(nj):
                                nc.tensor.matmul(
                                    psw[:, j * 512:j * 512 + T],
                                    wt_sb[:, kk,
                                          WOFF[k] + j * 128:
                                          WOFF[k] + (j + 1) * 128],
                                    x_sb[:, ci, kk, off:off + T],
                                    start=(kk == 0), stop=(kk == K_TILES - 1))
                        if evict == "none":
                            continue
                        sl = ws_chunk[k] % 2
                        ws_chunk[k] += 1
                        for j in range(nj):
                            dst = o_ws[k][:, sl, j * 512:j * 512 + T]
                            if evict == "copy":
                                nc.vector.tensor_copy(
                                    dst, psw[:, j * 512:j * 512 + T])
                            else:
                                nc.vector.tensor_scalar_add(
                                    dst, psw[:, j * 512:j * 512 + T],
                                    bbt_sb[:, wsj_col[(k, j)]:
                                           wsj_col[(k, j)] + 1])
                            if drop_out:
                                continue
                            eng = nc.scalar if FLUSH_ACT else nc.sync
                            eng.dma_start(
                                outs[k][j * 128:(j + 1) * 128,
                                        tloc:tloc + T], dst)
                        continue
                    bi = (tok - seg_start[k]) // 128
                    slot = bi % P
                    if slot == 0:
                        cur_ps = psp.tile([128, 1024], f32, tag="ps")
                    ps = cur_ps
                    # kk outer / column-chunk inner: consecutive matmuls in
                    # a >512-wide expert reuse the same stationary x tile
                    # (~17 ns/matmul cheaper on HW than swapping it)
                    if KK_OUTER:
                        for kk in range(K_TILES):
                            for j0 in range(0, ok, 512):
                                jn = min(512, ok - j0)
                                nc.tensor.matmul(
                                    ps[:, slot * ok + j0:slot * ok + j0 + jn],
                                    x_sb[:, ci, kk, off:off + 128],
                                    wt_sb[:, kk,
                                          WOFF[k] + j0:WOFF[k] + j0 + jn],
                                    start=(kk == 0), stop=(kk == K_TILES - 1))
                    else:
                        for j0 in range(0, ok, 512):
                            jn = min(512, ok - j0)
                            for kk in range(K_TILES):
                                nc.tensor.matmul(
                                    ps[:, slot * ok + j0:slot * ok + j0 + jn],
                                    x_sb[:, ci, kk, off:off + 128],
                                    wt_sb[:, kk,
                                          WOFF[k] + j0:WOFF[k] + j0 + jn],
                                    start=(kk == 0), stop=(kk == K_TILES - 1))
                    if slot != P - 1 or evict == "none":
                        continue
                    h = o_half[k]
                    b0 = bi - slot
                    dst = o_seg[k][:, b0 % h:b0 % h + P, :] \
                        .rearrange("p j n -> p (j n)")
                    if evict == "copy":
                        nc.vector.tensor_copy(dst, ps[:, :P * ok])
                    else:
                        bias = (bbr_sb[:, bbr_off[k]:bbr_off[k] + P * ok]
                                if P > 1 else bb_sb[:, WOFF[k]:WOFF[k] + ok])
                        nc.vector.tensor_add(dst, ps[:, :P * ok], bias)
                    if drop_out:
                        continue
                    # Fine-grained flushes on the Activation-engine HWDGE
                    # ring: never queued behind the prologue x loads on the
                    # SP ring (that FIFO stalled the PSUM recycle chain
                    # ~11 us), and the final flush is a ~0.3-0.5 MB chunk
                    # instead of a half-segment (short drain tail). A flush
                    # fires every FB blocks and always at a slot-wrap or the
                    # segment end, so chunks never straddle the o_seg wrap.
                    nblk_k = caps[k] // 128
                    eng = nc.scalar if FLUSH_ACT else nc.sync
                    if not FLUSH_FINE:
                        if bi == h - 1 and h < nblk_k:
                            eng.dma_start(
                                outs[k][:h * 128]
                                .rearrange("(j p) n -> p j n", p=128),
                                o_seg[k][:])
                        elif bi == nblk_k - 1:
                            lo = h * 128 if h < nblk_k else 0
                            eng.dma_start(
                                outs[k][lo:]
                                .rearrange("(j p) n -> p j n", p=128),
                                o_seg[k][:, :nblk_k - (lo // 128)])
                    elif (bi + 1 - fl_next[k] >= FB[k] or (bi + 1) % h == 0
                            or bi == nblk_k - 1):
                        b0 = fl_next[k]
                        nf = bi + 1 - b0
                        eng.dma_start(
                            outs[k][b0 * 128:(bi + 1) * 128]
                            .rearrange("(j p) n -> p j n", p=128),
                            o_seg[k][:, b0 % h:b0 % h + nf])
                        fl_next[k] = bi + 1

            if loop:
                with tc.For_i(0, loop, 1):
                    body()
            else:
                for _ in range(repeat):
                    body()
    nc.compile()
    return nc


def _get_nc(caps, repeat=1, loop=None):
    key = (tuple(caps), repeat, loop)
    if key not in _nc_cache:
        _nc_cache[key] = _build(caps, repeat=repeat, loop=loop)
    return _nc_cache[key]


def _route(out_feat_size):
    """Map out_feat_size values -> expert index (-1 = matches no expert)."""
    ofs = np.asarray(out_feat_size).astype(np.int64).reshape(-1)
    branch = np.full(ofs.shape, -1, dtype=np.int64)
    for k, s in enumerate(OUT_SIZES):
        branch[ofs == s] = k
    return branch


def _plan(branch):
    """Balanced routing plan: per-expert global index lists split evenly
    across cores, shared capacities, and segment layout [3,2,1,0]."""
    idx_all = {k: np.nonzero(branch == k)[0] for k in range(N_EXPERTS)}
    per_core = [int(-(-len(idx_all[k]) // N_CORES)) for k in range(N_EXPERTS)]
    caps = [int(-(-per_core[k] // 128) * 128) for k in range(N_EXPERTS)]
    return idx_all, tuple(caps)


def kernel(x, weight, bias, out_feat_size):
    import ml_dtypes
    from concourse.bass_utils import run_bass_kernel_spmd

    bf16 = ml_dtypes.bfloat16
    x = np.asarray(x, dtype=np.float32)
    weight = np.asarray(weight, dtype=np.float32)
    bias = np.asarray(bias, dtype=np.float32)
    B, T, D = x.shape
    assert D == IN_FEAT
    n_tok = B * T

    branch = _route(out_feat_size)
    idx_all, caps = _plan(branch)
    if sum(caps) == 0:
        return np.zeros((B, T, IN_FEAT), dtype=np.float32)

    # host-side weight/bias layout
    wt = np.empty((IN_FEAT, W_COLS), dtype=bf16)
    bb = np.empty((W_COLS,), dtype=np.float32)
    for k, ok in enumerate(OUT_SIZES):
        wt[:, WOFF[k]:WOFF[k] + ok] = weight[k, :ok, :].T.astype(bf16)
        bb[WOFF[k]:WOFF[k] + ok] = bias[k, :ok]
    bb128 = np.ascontiguousarray(np.broadcast_to(bb.astype(bf16),
                                                 (128, W_COLS)))
    bbr = np.concatenate([np.tile(bb[WOFF[0]:WOFF[0] + 128], 4),
                          np.tile(bb[WOFF[1]:WOFF[1] + 256], 2)])
    bbr128 = np.ascontiguousarray(np.broadcast_to(bbr.astype(bf16),
                                                  (128, 1024)))
    ws = [k for k in range(N_EXPERTS)
          if WS_SMALL and OUT_SIZES[k] <= 256 and caps[k]]
    bbt_cols = [bias[k, j * 128:(j + 1) * 128]
                for k in ws for j in range(OUT_SIZES[k] // 128)]
    bbt = (np.ascontiguousarray(np.stack(bbt_cols, axis=1)
                                .astype(np.float32))
           if bbt_cols else None)

    x2 = x.reshape(n_tok, IN_FEAT).astype(bf16)
    tpad = sum(caps)
    nblocks = -(-tpad // BLK)
    tdma = nblocks * BLK
    seg_off = {}
    t0 = 0
    for k in SEG_ORDER:
        if caps[k]:
            seg_off[k] = t0
            t0 += caps[k]

    in_maps = []
    core_slices = []  # per core: {expert: global idx array}
    for c in range(N_CORES):
        perm = np.zeros(tdma, dtype=np.int64)
        slices = {}
        for k, off in seg_off.items():
            idx = idx_all[k]
            m = int(-(-len(idx) // N_CORES))
            part = idx[c * m:(c + 1) * m]
            slices[k] = part
            if len(part):
                perm[off:off + len(part)] = part
                perm[off + len(part):off + caps[k]] = part[0]
        xtb = np.empty((nblocks, IN_FEAT, BLK), dtype=bf16)
        for ci in range(nblocks):
            np.copyto(xtb[ci], x2[perm[ci * BLK:(ci + 1) * BLK]].T)
        im = {"xt": xtb, "wt": wt, "bb": bb128, "bbr": bbr128}
        if bbt is not None:
            im["bbt"] = bbt
        in_maps.append(im)
        core_slices.append(slices)

    global _LAST_CAPS, _LAST_IN_MAPS
    _LAST_CAPS, _LAST_IN_MAPS = caps, in_maps

    nc = _get_nc(caps)
    res = run_bass_kernel_spmd(nc, in_maps, list(range(N_CORES))).results

    out = np.zeros((n_tok, IN_FEAT), dtype=np.float32)
    for c in range(N_CORES):
        for k, part in core_slices[c].items():
            n = len(part)
            if n == 0:
                continue
            ok = OUT_SIZES[k]
            if k in ws:
                out[part, :ok] = res[c][f"out{k}"][:, :n].T.astype(np.float32)
            else:
                out[part, :ok] = res[c][f"out{k}"][:n].astype(np.float32)
            if ok < IN_FEAT:
                # reference semantics: bias tail beyond out_k (zero for the
                # shipped inputs, which pre-zero the bias)
                out[part, ok:] = bias[k, ok:]
    return out.reshape(B, T, IN_FEAT)



# revision 32
# speedup vs baseline: 1.0890x; 1.0890x over previous
"""MultiOutSizeLinear (MoE-style routed linear) for Trainium2, 8 NeuronCores.

Each token selects one of 4 experts by its ``out_feat_size`` value
(128/256/512/1024). Expert k is a dense [out_k, 1024] linear + bias whose
output lands in the first out_k columns of the 1024-wide output row; the
reference leaves bias[k, out_k:] in the remaining columns (zero for the
shipped setup_inputs, which pre-zeroes the bias tail).

Strategy
  host:   route tokens to experts; balance each expert's tokens evenly
          across the 8 cores (capacities are shared so one SPMD program
          serves all cores); gather + transpose each core's tokens into
          x^T [1024, TPAD] laid out as expert segments [e3 | e2 | e1 | e0],
          cast to bfloat16 (fp32 PSUM accumulation keeps rel err ~1e-3).
  device: keep W^T [1024, 1920] (all experts, concatenated out-columns),
          bias tiles, small per-expert output staging buffers, and -- SBUF
          permitting, which it is for the shipped shapes -- the ENTIRE
          per-core x^T resident in SBUF, so the steady-state body issues
          ZERO read DMAs.

          Single-shot schedule (what a one-shot profile sees):
          - Prologue DMAs are issued just-in-time on the SP HWDGE ring:
            [wt(first expert) ⊕ x group 0, per ktile] bb bbr wt(2nd) x1 x2
            x3 wt(3rd) ... so the first matmul waits on ~0.4 MB, not 5 MB
            (head stall 19 us -> 3 us modeled).
          - Output flushes ride the Activation-engine HWDGE ring in
            0.13-0.5 MB chunks, so they never sit FIFO-behind prologue x
            loads (that stalled the PSUM recycle chain ~11 us) and the
            final drain tail is ~1 us.
          - Segments run big-expert-first [e3|e2|e1|e0]: e3's per-group PE
            time is the largest, so the x stream stays ahead of compute.

          PE schedule: HW measures ~0.63-0.74 ns/col sustained (power
          throttle off the 2.4 GHz nominal; drifts with load history) plus
          ~26 ns fixed cost per matmul instruction, so the instruction
          count matters as much as streamed columns:
          - e2/e3 token-stationary: psum[128 tok, out_k] += xT.T @ wT, kk
            OUTER / 512-col chunk inner, so e3's consecutive matmuls reuse
            the stationary x tile (~17 ns cheaper than swapping).
          - e0/e1 weight-stationary (WS_SMALL): stationary = w tile
            [128 k, 128 out-cols], moving = x [128 k, <=512 tokens] ->
            psum[128 out-col, tok]; 96 matmuls of 512-wide streams replace
            256 matmuls of 128/256-wide ones. Output is transposed
            ([out_k, cap]); the host untransposes for free. Bias comes
            from per-partition f32 columns (``bbt``) via tensor_scalar_add.
          Bias for e2/e3 is added on VectorE during PSUM eviction.
  host:   scatter rows back through the routing permutation (upcast to f32).

Measured (8-core SPMD, loop-delta steady state): ~177-210 us depending on
the chip's sustained-throttle state; CoreSim single-shot model 121 us at
2.4 GHz / 173 us at the observed 1.55 GHz sustained rate (baseline kernel:
147.5 / 191.3). fp8 DoubleRow was measured 2x bf16 on HW but plain-fp8
accuracy (rel ~3.8e-2) fails the 2e-2 gate and any error-compensation term
costs full moving-port bandwidth, erasing the speedup -- so bf16 it is.
"""

import sys
import numpy as np

sys.path.insert(0, "/opt/trn_rl_repo")

OUT_SIZES = (128, 256, 512, 1024)
N_EXPERTS = len(OUT_SIZES)
IN_FEAT = 1024
N_CORES = 8
K_TILES = IN_FEAT // 128
BLK = 512       # tokens per x^T DRAM block
XGRP = 1        # 512-token blocks per x dma_start
CHUNK = BLK     # kept name: host DRAM layout is BLK-blocked
WOFF = tuple(int(np.cumsum((0,) + OUT_SIZES)[k]) for k in range(N_EXPERTS))
W_COLS = sum(OUT_SIZES)
# Big expert first: its per-group PE time is the largest, so the x-group
# stream on the SP ring never starves the compute cursor; the smaller
# experts run at the end when all of x is long resident. (Small-first
# reads ~200 KB before the first matmul but starves mid-kernel: modeled
# 198 us vs 174 us at the sustained HW clock.)
SEG_ORDER = (3, 2, 1, 0)   # segment order along the token axis
KK_OUTER = True            # kk outer / col-chunk inner for >512-wide experts
FLUSH_FINE = True          # fine-grained output flushes vs half-segment
FLUSH_ACT = True           # output flushes on the Activation HWDGE ring
WS_SMALL = True            # weight-stationary path for experts <=256 wide:
                           # 512-token moving streams instead of 128/256-col
                           # ones cut the matmul count 256 -> 96 (~26 ns of
                           # fixed issue cost each); output comes out
                           # transposed [ok, cap] and is fixed up on host

_nc_cache: dict = {}


def _build(caps, repeat=1, loop=None, xbufs=3, obufs=None,
           drop_out=False, fake_x=False, resident_groups="auto",
           read_lead=None, evict="add"):
    """Compile the SPMD program for shared per-expert capacities ``caps``.

    caps[k] % 128 == 0; computed tokens sum(caps) need not be 512-aligned
    (the final DRAM block is padded). ``repeat``/``loop`` re-run the
    compute body (same I/O) for timing.
    """
    import concourse.bacc as bacc
    import concourse.mybir as mybir
    import concourse.tile as tile

    f32 = mybir.dt.float32
    bf16 = mybir.dt.bfloat16
    ws_set = {k for k in range(N_EXPERTS)
              if WS_SMALL and OUT_SIZES[k] <= 256 and caps[k]}
    tpad = sum(caps)
    assert all(c % 128 == 0 for c in caps)
    nblocks = -(-tpad // BLK)          # DRAM blocks (last may be partial)
    ngroups = -(-nblocks // XGRP)

    if resident_groups == "auto":
        # resident x^T groups: whatever SBUF has left after the weights,
        # bias, and per-expert half-segment output buffers. When the whole
        # x^T fits (the common case), the body has ZERO read DMAs and the
        # streaming rotation pool is never used.
        grp_b = XGRP * K_TILES * BLK * 2
        fixed = (K_TILES * W_COLS * 2 + W_COLS * 2
                 + sum(-(-(caps[k] // 128) // 2) * OUT_SIZES[k] * 2
                       for k in range(N_EXPERTS)))
        if fixed + ngroups * grp_b <= 206 * 1024:
            resident_groups = ngroups
        else:
            resident_groups = max(0, (200 * 1024 - fixed - xbufs * grp_b)
                                  // grp_b)

    nc = bacc.Bacc(None, target_bir_lowering=False, debug=False)
    # block-layout x^T: block c holds tokens [c*BLK, (c+1)*BLK) as a
    # contiguous [IN_FEAT, BLK] slab; one dma_start covers XGRP blocks
    xt = nc.dram_tensor("xt", [nblocks, IN_FEAT, BLK], bf16,
                        kind="ExternalInput")
    wt = nc.dram_tensor("wt", [IN_FEAT, W_COLS], bf16, kind="ExternalInput")
    bb = nc.dram_tensor("bb", [128, W_COLS], bf16, kind="ExternalInput")
    # replicated bias for packed evictions: cols [0,512) = bias0 x4,
    # cols [512,1024) = bias1 x2
    bbr = nc.dram_tensor("bbr", [128, 1024], bf16, kind="ExternalInput")
    # per-partition bias columns for the weight-stationary experts:
    # bbt[p, wsj_col[(k, j)]] = bias[k, j*128 + p]
    wsj_col = {}
    for k in sorted(ws_set):
        for j in range(OUT_SIZES[k] // 128):
            wsj_col[(k, j)] = len(wsj_col)
    bbt = (nc.dram_tensor("bbt", [128, len(wsj_col)], f32,
                          kind="ExternalInput") if wsj_col else None)
    outs = {k: nc.dram_tensor(
                f"out{k}",
                ([OUT_SIZES[k], caps[k]] if k in ws_set
                 else [caps[k], OUT_SIZES[k]]), bf16,
                kind="ExternalOutput")
            for k in range(N_EXPERTS) if caps[k]}

    # smallest expert first: its weight slice + first x tiles are a few
    # hundred KB, so the PE starts ~2 us into the kernel instead of waiting
    # for the big experts' multi-MB weight slabs; each later expert's wt
    # chunk is DMA'd while earlier segments compute
    seg_order = [k for k in SEG_ORDER if caps[k] > 0]
    seg_start = {}
    t0 = 0
    for k in seg_order:
        seg_start[k] = t0
        t0 += caps[k]

    def expert_of(tok):
        for k in seg_order:
            if tok < seg_start[k] + caps[k]:
                return k
        raise AssertionError

    with tile.TileContext(nc) as tc:
        with (
            tc.tile_pool(name="const", bufs=1) as const,
            tc.tile_pool(name="xp", bufs=xbufs) as xp,
            tc.tile_pool(name="ps", bufs=4, space="PSUM") as psp,
        ):
            # Prologue DMA order is just-in-time for the compute schedule:
            #   [wt(e_first) ⊕ x group 0 per-ktile] bb bbr wt(e2nd) x1 x2 x3
            #   wt(e3rd) x4 x5 wt(e4th) x6 ... x16
            # so the first matmul waits on ~0.2 MB, and every later expert's
            # weight slice / x group lands before its segment needs it.
            wt_sb = const.tile([128, K_TILES, W_COLS], bf16)
            xr0 = None
            g0 = tk0 = 0
            if resident_groups != 0 and ngroups > 0:
                xr0 = const.tile([128, XGRP, K_TILES, BLK], bf16, name="xres0")
                g0 = min(XGRP, nblocks)
                tk0 = BLK if (nblocks > 1 or tpad % BLK == 0) else tpad % BLK
            k1 = seg_order[0]
            for kk in range(K_TILES):
                nc.sync.dma_start(
                    wt_sb[:, kk, WOFF[k1]:WOFF[k1] + OUT_SIZES[k1]],
                    wt[kk * 128:(kk + 1) * 128,
                       WOFF[k1]:WOFF[k1] + OUT_SIZES[k1]])
                if xr0 is not None:
                    for ci in range(g0):
                        tk = tk0 if ci == g0 - 1 else BLK
                        nc.sync.dma_start(
                            xr0[:, ci, kk, :tk],
                            xt[ci, kk * 128:(kk + 1) * 128, :tk])
            bb_sb = const.tile([128, W_COLS], bf16)
            nc.sync.dma_start(bb_sb[:], bb[:])
            bbr_sb = const.tile([128, 1024], bf16)
            nc.sync.dma_start(bbr_sb[:], bbr[:])
            bbr_off = {0: 0, 1: 512}

            def load_wt_expert(k):
                for kk in range(K_TILES):
                    nc.sync.dma_start(
                        wt_sb[:, kk, WOFF[k]:WOFF[k] + OUT_SIZES[k]],
                        wt[kk * 128:(kk + 1) * 128,
                           WOFF[k]:WOFF[k] + OUT_SIZES[k]])
            # remaining experts' weights interleave with the x-group stream,
            # each issued ~2 groups before its segment starts: early enough
            # to land in time, late enough not to delay the x groups the
            # compute cursor needs first (a frontloaded wt_e2 before group 1
            # stalled the PE 5.3 us at full clock)
            wt_before_group = {}
            for k in seg_order[1:]:
                slot = max(1, seg_start[k] // (XGRP * BLK) - 2)
                while slot in wt_before_group:
                    slot += 1
                wt_before_group[slot] = k

            # half-segment output buffers: DVE evicts into slot bi % half;
            # flushed as ONE DMA per half per expert per iteration (the
            # 4-deep PSUM rotation absorbs the WAR wait when the second
            # half starts while the first half's flush drains)
            o_seg = {}
            o_half = {}
            odt = mybir.dt.float32 if evict == "addf32" else bf16
            for k in seg_order:
                if k in ws_set:
                    continue
                nblk_k = caps[k] // 128
                o_half[k] = 4 if drop_out else -(-nblk_k // 2)
                o_seg[k] = const.tile([128, o_half[k], OUT_SIZES[k]],
                                      odt, name=f"oseg{k}")
            # weight-stationary experts: transposed [out-col part, token]
            # chunk buffers, double-buffered against the flush DMA
            o_ws = {k: const.tile([128, 2, (OUT_SIZES[k] // 128) * 512],
                                  odt, name=f"ows{k}")
                    for k in sorted(ws_set)}
            bbt_sb = None
            if wsj_col:
                bbt_sb = const.tile([128, len(wsj_col)], f32)
                nc.sync.dma_start(bbt_sb[:], bbt[:])

            # blocks packed per PSUM tile (ok*pack <= 512): batches expert
            # 0/1 evictions so the PE->DVE->PE psum-recycle sem chain fires
            # 4x/2x less often in the fast small-expert tail
            packs = {}
            for k in seg_order:
                if k in ws_set:
                    packs[k] = 1
                    continue
                nblk_k = caps[k] // 128
                p = 1
                if k in bbr_off:
                    for cand in (4, 2):
                        if (cand * OUT_SIZES[k] <= 512
                                and nblk_k % cand == 0
                                and o_half[k] % cand == 0):
                            p = cand
                            break
                packs[k] = p

            def xsrc(gi):
                """(blocks, tokens-in-last-block, src AP) for x group gi.
                The final DRAM block is only read up to the last computed
                token."""
                s = gi * XGRP
                g = min(XGRP, nblocks - s)
                tk = BLK
                if s + g == nblocks and tpad % BLK and g == 1:
                    tk = tpad % BLK
                    src = xt[s:s + 1, :, :tk].rearrange(
                        "c (kk p) t -> p c kk t", p=128)
                else:
                    src = xt[s:s + g].rearrange("c (kk p) t -> p c kk t", p=128)
                return g, tk, src

            # leading groups of x^T stay resident: each loop iteration
            # starts computing on them immediately (loaded once, before the
            # loop), and they are not re-read per iteration. Group 0 was
            # already issued (per-ktile, interleaved with wt) above.
            nres = min(resident_groups, ngroups)
            xres = {}
            if xr0 is not None and nres > 0:
                xres[0] = xr0
            for gi in range(1, nres):
                if gi in wt_before_group:
                    load_wt_expert(wt_before_group[gi])
                gsz, tk, src = xsrc(gi)
                xr = const.tile([128, XGRP, K_TILES, BLK], bf16,
                                name=f"xres{gi}")
                nc.sync.dma_start(xr[:, :gsz, :, :tk], src)
                xres[gi] = xr
            # wt slices not yet issued (few/zero resident groups)
            for gi, k in wt_before_group.items():
                if gi >= max(nres, 1):
                    load_wt_expert(k)

            # flush cadence (blocks): >=2 KB per partition line per DMA,
            # always a multiple of the pack size, and <= the slot count so
            # a flush is issued before any o_seg slot is reused
            FB = {k: min(max(packs[k], 2048 // (OUT_SIZES[k] * 2) or 1),
                         o_half[k])
                  for k in seg_order if k not in ws_set}

            def body():
                xtiles = dict(xres)
                fl_next = {k: 0 for k in seg_order}
                ws_chunk = {k: 0 for k in ws_set}

                def issue_x(gi):
                    if gi < nres or gi >= ngroups or fake_x:
                        return
                    g, tk, src = xsrc(gi)
                    x_sb = xp.tile([128, XGRP, K_TILES, BLK], bf16, tag="x")
                    nc.sync.dma_start(x_sb[:, :g, :, :tk], src)
                    xtiles[gi] = x_sb

                if read_lead is None:
                    for gi in range(nres, ngroups):
                        issue_x(gi)
                else:
                    for gi in range(nres, min(nres + read_lead, ngroups)):
                        issue_x(gi)

                next_read = nres + (read_lead or 0)
                cur_ps = None
                for tok in range(0, tpad, 128):
                    k = expert_of(tok)
                    ok = OUT_SIZES[k]
                    P = packs[k]
                    blk, off = divmod(tok, BLK)
                    gi, ci = divmod(blk, XGRP)
                    if read_lead is not None and off == 0 and ci == 0:
                        # compute cursor entered group gi: top up the lead
                        while next_read <= gi + read_lead and next_read < ngroups:
                            issue_x(next_read)
                            next_read += 1
                    x_sb = xtiles[0 if fake_x else gi]
                    if k in ws_set:
                        # weight-stationary: one [128-out-col, T<=512-token]
                        # chunk per psum group, streaming tokens as the
                        # moving operand. Emitted once per chunk (chunks
                        # split at xt-block boundaries); other 128-blocks
                        # inside the chunk are skipped.
                        tloc = tok - seg_start[k]
                        if tloc != 0 and off != 0:
                            continue
                        T = min(BLK - off, caps[k] - tloc)
                        nj = ok // 128
                        psw = psp.tile([128, 1024], f32, tag="ps")
                        for kk in range(K_TILES):
                            for j in range(nj):
                                nc.tensor.matmul(
                                    psw[:, j * 512:j * 512 + T],
                                    wt_sb[:, kk,
                                          WOFF[k] + j * 128:
                                          WOFF[k] + (j + 1) * 128],
                                    x_sb[:, ci, kk, off:off + T],
                                    start=(kk == 0), stop=(kk == K_TILES - 1))
                        if evict == "none":
                            continue
                        sl = ws_chunk[k] % 2
                        ws_chunk[k] += 1
                        for j in range(nj):
                            dst = o_ws[k][:, sl, j * 512:j * 512 + T]
                            if evict == "copy":
                                nc.vector.tensor_copy(
                                    dst, psw[:, j * 512:j * 512 + T])
                            else:
                                nc.vector.tensor_scalar_add(
                                    dst, psw[:, j * 512:j * 512 + T],
                                    bbt_sb[:, wsj_col[(k, j)]:
                                           wsj_col[(k, j)] + 1])
                            if drop_out:
                                continue
                            eng = nc.scalar if FLUSH_ACT else nc.sync
                            eng.dma_start(
                                outs[k][j * 128:(j + 1) * 128,
                                        tloc:tloc + T], dst)
                        continue
                    bi = (tok - seg_start[k]) // 128
                    slot = bi % P
                    if slot == 0:
                        cur_ps = psp.tile([128, 1024], f32, tag="ps")
                    ps = cur_ps
                    # kk outer / column-chunk inner: consecutive matmuls in
                    # a >512-wide expert reuse the same stationary x tile
                    # (~17 ns/matmul cheaper on HW than swapping it)
                    if KK_OUTER:
                        for kk in range(K_TILES):
                            for j0 in range(0, ok, 512):
                                jn = min(512, ok - j0)
                                nc.tensor.matmul(
                                    ps[:, slot * ok + j0:slot * ok + j0 + jn],
                                    x_sb[:, ci, kk, off:off + 128],
                                    wt_sb[:, kk,
                                          WOFF[k] + j0:WOFF[k] + j0 + jn],
                                    start=(kk == 0), stop=(kk == K_TILES - 1))
                    else:
                        for j0 in range(0, ok, 512):
                            jn = min(512, ok - j0)
                            for kk in range(K_TILES):
                                nc.tensor.matmul(
                                    ps[:, slot * ok + j0:slot * ok + j0 + jn],
                                    x_sb[:, ci, kk, off:off + 128],
                                    wt_sb[:, kk,
                                          WOFF[k] + j0:WOFF[k] + j0 + jn],
                                    start=(kk == 0), stop=(kk == K_TILES - 1))
                    if slot != P - 1 or evict == "none":
                        continue
                    h = o_half[k]
                    b0 = bi - slot
                    dst = o_seg[k][:, b0 % h:b0 % h + P, :] \
                        .rearrange("p j n -> p (j n)")
                    if evict == "copy":
                        nc.vector.tensor_copy(dst, ps[:, :P * ok])
                    else:
                        bias = (bbr_sb[:, bbr_off[k]:bbr_off[k] + P * ok]
                                if P > 1 else bb_sb[:, WOFF[k]:WOFF[k] + ok])
                        nc.vector.tensor_add(dst, ps[:, :P * ok], bias)
                    if drop_out:
                        continue
                    # Fine-grained flushes on the Activation-engine HWDGE
                    # ring: never queued behind the prologue x loads on the
                    # SP ring (that FIFO stalled the PSUM recycle chain
                    # ~11 us), and the final flush is a ~0.3-0.5 MB chunk
                    # instead of a half-segment (short drain tail). A flush
                    # fires every FB blocks and always at a slot-wrap or the
                    # segment end, so chunks never straddle the o_seg wrap.
                    nblk_k = caps[k] // 128
                    eng = nc.scalar if FLUSH_ACT else nc.sync
                    if not FLUSH_FINE:
                        if bi == h - 1 and h < nblk_k:
                            eng.dma_start(
                                outs[k][:h * 128]
                                .rearrange("(j p) n -> p j n", p=128),
                                o_seg[k][:])
                        elif bi == nblk_k - 1:
                            lo = h * 128 if h < nblk_k else 0
                            eng.dma_start(
                                outs[k][lo:]
                                .rearrange("(j p) n -> p j n", p=128),
                                o_seg[k][:, :nblk_k - (lo // 128)])
                    elif (bi + 1 - fl_next[k] >= FB[k] or (bi + 1) % h == 0
                            or bi == nblk_k - 1):
                        b0 = fl_next[k]
                        nf = bi + 1 - b0
                        eng.dma_start(
                            outs[k][b0 * 128:(bi + 1) * 128]
                            .rearrange("(j p) n -> p j n", p=128),
                            o_seg[k][:, b0 % h:b0 % h + nf])
                        fl_next[k] = bi + 1

            if loop:
                with tc.For_i(0, loop, 1):
                    body()
            else:
                for _ in range(repeat):
                    body()
    nc.compile()
    return nc


def _get_nc(caps, repeat=1, loop=None):
    key = (tuple(caps), repeat, loop)
    if key not in _nc_cache:
        _nc_cache[key] = _build(caps, repeat=repeat, loop=loop)
    return _nc_cache[key]


def _route(out_feat_size):
    """Map out_feat_size values -> expert index (-1 = matches no expert)."""
    ofs = np.asarray(out_feat_size).astype(np.int64).reshape(-1)
    branch = np.full(ofs.shape, -1, dtype=np.int64)
    for k, s in enumerate(OUT_SIZES):
        branch[ofs == s] = k
    return branch


def _plan(branch):
    """Balanced routing plan: per-expert global index lists split evenly
    across cores, shared capacities, and segment layout [3,2,1,0]."""
    idx_all = {k: np.nonzero(branch == k)[0] for k in range(N_EXPERTS)}
    per_core = [int(-(-len(idx_all[k]) // N_CORES)) for k in range(N_EXPERTS)]
    caps = [int(-(-per_core[k] // 128) * 128) for k in range(N_EXPERTS)]
    return idx_all, tuple(caps)


def kernel(x, weight, bias, out_feat_size):
    import ml_dtypes
    from concourse.bass_utils import run_bass_kernel_spmd

    bf16 = ml_dtypes.bfloat16
    x = np.asarray(x, dtype=np.float32)
    weight = np.asarray(weight, dtype=np.float32)
    bias = np.asarray(bias, dtype=np.float32)
    B, T, D = x.shape
    assert D == IN_FEAT
    n_tok = B * T

    branch = _route(out_feat_size)
    idx_all, caps = _plan(branch)
    if sum(caps) == 0:
        return np.zeros((B, T, IN_FEAT), dtype=np.float32)

    # host-side weight/bias layout
    wt = np.empty((IN_FEAT, W_COLS), dtype=bf16)
    bb = np.empty((W_COLS,), dtype=np.float32)
    for k, ok in enumerate(OUT_SIZES):
        wt[:, WOFF[k]:WOFF[k] + ok] = weight[k, :ok, :].T.astype(bf16)
        bb[WOFF[k]:WOFF[k] + ok] = bias[k, :ok]
    bb128 = np.ascontiguousarray(np.broadcast_to(bb.astype(bf16),
                                                 (128, W_COLS)))
    bbr = np.concatenate([np.tile(bb[WOFF[0]:WOFF[0] + 128], 4),
                          np.tile(bb[WOFF[1]:WOFF[1] + 256], 2)])
    bbr128 = np.ascontiguousarray(np.broadcast_to(bbr.astype(bf16),
                                                  (128, 1024)))
    ws = [k for k in range(N_EXPERTS)
          if WS_SMALL and OUT_SIZES[k] <= 256 and caps[k]]
    bbt_cols = [bias[k, j * 128:(j + 1) * 128]
                for k in ws for j in range(OUT_SIZES[k] // 128)]
    bbt = (np.ascontiguousarray(np.stack(bbt_cols, axis=1)
                                .astype(np.float32))
           if bbt_cols else None)

    x2 = x.reshape(n_tok, IN_FEAT).astype(bf16)
    tpad = sum(caps)
    nblocks = -(-tpad // BLK)
    tdma = nblocks * BLK
    seg_off = {}
    t0 = 0
    for k in SEG_ORDER:
        if caps[k]:
            seg_off[k] = t0
            t0 += caps[k]

    in_maps = []
    core_slices = []  # per core: {expert: global idx array}
    for c in range(N_CORES):
        perm = np.zeros(tdma, dtype=np.int64)
        slices = {}
        for k, off in seg_off.items():
            idx = idx_all[k]
            m = int(-(-len(idx) // N_CORES))
            part = idx[c * m:(c + 1) * m]
            slices[k] = part
            if len(part):
                perm[off:off + len(part)] = part
                perm[off + len(part):off + caps[k]] = part[0]
        xtb = np.empty((nblocks, IN_FEAT, BLK), dtype=bf16)
        for ci in range(nblocks):
            np.copyto(xtb[ci], x2[perm[ci * BLK:(ci + 1) * BLK]].T)
        im = {"xt": xtb, "wt": wt, "bb": bb128, "bbr": bbr128}
        if bbt is not None:
            im["bbt"] = bbt
        in_maps.append(im)
        core_slices.append(slices)

    global _LAST_CAPS, _LAST_IN_MAPS
    _LAST_CAPS, _LAST_IN_MAPS = caps, in_maps

    nc = _get_nc(caps)
    res = run_bass_kernel_spmd(nc, in_maps, list(range(N_CORES))).results

    out = np.zeros((n_tok, IN_FEAT), dtype=np.float32)
    for c in range(N_CORES):
        for k, part in core_slices[c].items():
            n = len(part)
            if n == 0:
                continue
            ok = OUT_SIZES[k]
            if k in ws:
                out[part, :ok] = res[c][f"out{k}"][:, :n].T.astype(np.float32)
            else:
                out[part, :ok] = res[c][f"out{k}"][:n].astype(np.float32)
            if ok < IN_FEAT:
                # reference semantics: bias tail beyond out_k (zero for the
                # shipped inputs, which pre-zero the bias)
                out[part, ok:] = bias[k, ok:]
    return out.reshape(B, T, IN_FEAT)

